# revision 1
# baseline (speedup 1.0000x reference)
"""Trainium2 Bass kernel for nn_BroadcastEdgeUpdate.

reference computes:
    res_edge_index = flat_atom_res_index[edge_index]           # [2, E]
    flatish_z      = z.reshape(R, n_res, c_z)                  # R = n_batch*n_res
    update         = einsum('rsc,ac->rsa', LN(flatish_z), W)   # [R, n_res, 16]
    out            = update[res_edge_index[0], res_edge_index[1] % n_res]

Sharding: core i owns table rows r0 in [64*i, 64*i+64) (z first-dim shard).
Edges are bucketed on the host by r0-block so each core gathers only from
its own locally-computed 2 MB table slice; the host undoes the permutation.

Device pipeline per core:
  phase A: z slice [32768, 128] --DMA--> bn_stats (DVE) -> rstd (ACT sqrt +
           DVE recip) -> fused (x-mu)*r (DVE tensor_scalar) -> PE transpose ->
           PE matmul with Wg = gamma*W^T -> PE transpose back -> +beta@W^T ->
           row-major [32768, 16] table in DRAM
  phase B: indirect-DMA gather, one descriptor per partition per
           instruction (walrus consumes one offset per partition; each
           descriptor copies a contiguous run). The host pairs edges whose
           table rows are (2k, 2k+1) so one descriptor serves two edges
           (a 128 B run); leftovers go through single-row instructions.
           352 pair insts + 336 single insts = 688 vs 1024 unpaired.
           (dma_gather/DMAGatherAnt would do 8k rows/inst but silently moves
           no data on this axon/fake_nrt runtime; ap_gather works but runs
           ~110 ns/idx on the Q7s — both rejected.)
"""

import numpy as np

import concourse.bass as bass
import concourse.bacc as bacc
import concourse.mybir as mybir
import concourse.tile as tile
from concourse import bass_utils
from concourse.bass import IndirectOffsetOnAxis

N_CORES = 8
N_RES = 512
C_Z = 128
C_AP = 16
ROWS_PER_CORE = (N_RES // N_CORES) * N_RES  # 32768 table rows
K_QUAD = 128                                # quad-gather insts (4 rows/descriptor)
K_PAIR = 128                                # pair-gather insts (2 rows/descriptor)
K_SING = 320                                # single-gather insts
QUAD_CAP = K_QUAD * 128                     # 16384 quads
PAIR_CAP = K_PAIR * 128                     # 16384 pairs
SING_CAP = K_SING * 128                     # 40960 singles
GB = 16                                     # gather insts batched per output DMA
SG_ROWS = 4096                              # rows per super-group (32 tiles)
N_SG = ROWS_PER_CORE // SG_ROWS             # 8
LN_EPS = 1e-5
DEBUG_TABLE = False

_prog_cache = {}


def _build_program():
    f32 = mybir.dt.float32
    i32 = mybir.dt.int32
    nc = bacc.Bacc("TRN2", target_bir_lowering=False, debug=False,
                   num_devices=N_CORES)

    zs = nc.dram_tensor("zs", [ROWS_PER_CORE, C_Z], f32, kind="ExternalInput").ap()
    wg = nc.dram_tensor("wg", [C_Z, C_AP], f32, kind="ExternalInput").ap()
    bw64 = nc.dram_tensor("bw64", [128, 4 * C_AP], f32, kind="ExternalInput").ap()
    ident = nc.dram_tensor("ident", [128, 128], f32, kind="ExternalInput").ap()
    eidx = nc.dram_tensor("eidx", [128, K_QUAD + K_PAIR + K_SING], i32,
                          kind="ExternalInput").ap()
    # quads, then pairs, then singles; slot j = k*128 + p in each region
    out = nc.dram_tensor(
        "out", [128, (4 * K_QUAD + 2 * K_PAIR + K_SING) * C_AP], f32,
        kind="ExternalOutput").ap()
    table_dbg = (nc.dram_tensor("table_dbg", [ROWS_PER_CORE, C_AP], f32,
                                kind="ExternalOutput").ap()
                 if DEBUG_TABLE else None)

    with tile.TileContext(nc) as tc:
        with (
            tc.tile_pool(name="const", bufs=1) as cpool,
            tc.tile_pool(name="xin", bufs=2) as xpool,
            tc.tile_pool(name="xn", bufs=2) as xnpool,
            tc.tile_pool(name="stat", bufs=2) as spool,
            tc.tile_pool(name="xnt", bufs=3) as tpool,
            tc.tile_pool(name="u", bufs=3) as upool,
            tc.tile_pool(name="ostage", bufs=2) as opool,
            tc.tile_pool(name="psumT", bufs=2, space="PSUM") as ptpool,
            tc.tile_pool(name="psumA", bufs=2, space="PSUM") as papool,
            tc.tile_pool(name="psum2", bufs=2, space="PSUM") as p2pool,
            tc.tile_pool(name="gidx", bufs=1) as gipool,
            tc.tile_pool(name="gout", bufs=4) as gopool,
            tc.tile_pool(name="tbl", bufs=1, space="DRAM") as dpool,
        ):
            wg_t = cpool.tile([C_Z, C_AP], f32)
            nc.sync.dma_start(out=wg_t[:], in_=wg[:, :])
            bw_t = cpool.tile([128, 4 * C_AP], f32)
            nc.sync.dma_start(out=bw_t[:], in_=bw64[:, :])
            id_t = cpool.tile([128, 128], f32)
            nc.sync.dma_start(out=id_t[:], in_=ident[:, :])

            table = dpool.tile([ROWS_PER_CORE, C_AP], f32)

            # ---------------- phase A: build the update table ----------------
            for sg in range(N_SG):
                x = xpool.tile([128, 32, C_Z], f32, tag="x")
                rows = zs[sg * SG_ROWS:(sg + 1) * SG_ROWS, :]
                nc.sync.dma_start(out=x[:], in_=rows.rearrange("(t p) c -> p t c", p=128))

                stats = spool.tile([128, 32, 6], f32, tag="stats")
                for t in range(32):
                    nc.vector.bn_stats(out=stats[:, t, :], in_=x[:, t, :])

                # combine even/odd stats: n=128, ce=co=64
                # var = (M2e + M2o + 32*(me-mo)^2)/128 ; mean = (me+mo)/2
                t1 = spool.tile([128, 32, 1], f32, tag="t1")
                t2 = spool.tile([128, 32, 1], f32, tag="t2")
                t3 = spool.tile([128, 32, 1], f32, tag="t3")
                sd = spool.tile([128, 32, 1], f32, tag="sd")
                rr = spool.tile([128, 32, 1], f32, tag="rr")
                ms = spool.tile([128, 32, 1], f32, tag="ms")
                nc.vector.tensor_tensor(out=t1[:], in0=stats[:, :, 1:2],
                                        in1=stats[:, :, 4:5],
                                        op=mybir.AluOpType.subtract)
                nc.vector.tensor_tensor(out=t2[:], in0=stats[:, :, 2:3],
                                        in1=stats[:, :, 5:6],
                                        op=mybir.AluOpType.add)
                nc.vector.tensor_tensor(out=t3[:], in0=t1[:], in1=t1[:],
                                        op=mybir.AluOpType.mult)
                # t3 <- 32*t3 + 128*eps, then += t2  == 128*(var + eps)
                nc.vector.tensor_scalar(out=t3[:], in0=t3[:], scalar1=32.0,
                                        scalar2=float(C_Z * LN_EPS),
                                        op0=mybir.AluOpType.mult,
                                        op1=mybir.AluOpType.add)
                nc.vector.tensor_tensor(out=t3[:], in0=t3[:], in1=t2[:],
                                        op=mybir.AluOpType.add)
                nc.scalar.activation(out=sd[:], in_=t3[:],
                                     func=mybir.ActivationFunctionType.Sqrt,
                                     bias=0.0, scale=1.0 / C_Z)
                nc.vector.reciprocal(out=rr[:], in_=sd[:])
                nc.vector.tensor_tensor(out=ms[:], in0=stats[:, :, 1:2],
                                        in1=stats[:, :, 4:5],
                                        op=mybir.AluOpType.add)
                nc.vector.tensor_scalar(out=ms[:], in0=ms[:], scalar1=0.5,
                                        scalar2=None, op0=mybir.AluOpType.mult)

                xn = xnpool.tile([128, 32, C_Z], f32, tag="xn")
                for t in range(32):
                    nc.vector.tensor_scalar(out=xn[:, t, :], in0=x[:, t, :],
                                            scalar1=ms[:, t, :],
                                            scalar2=rr[:, t, :],
                                            op0=mybir.AluOpType.subtract,
                                            op1=mybir.AluOpType.mult)

                ostage = opool.tile([128, 32, C_AP], f32, tag="ostage")
                for gg in range(8):
                    psum_t = ptpool.tile([128, 512], f32, tag="pt")
                    for t4 in range(4):
                        nc.tensor.transpose(out=psum_t[:, 128 * t4:128 * (t4 + 1)],
                                            in_=xn[:, 4 * gg + t4, :],
                                            identity=id_t[:])
                    xnt = tpool.tile([128, 512], f32, tag="xnt")
                    if gg % 2 == 0:
                        nc.vector.tensor_copy(out=xnt[:], in_=psum_t[:])
                    else:
                        nc.scalar.copy(out=xnt[:], in_=psum_t[:])
                    psum_a = papool.tile([C_AP, 512], f32, tag="pa")
                    nc.tensor.matmul(out=psum_a[:], lhsT=wg_t[:], rhs=xnt[:],
                                     start=True, stop=True)
                    u = upool.tile([C_AP, 512], f32, tag="u")
                    nc.scalar.copy(out=u[:], in_=psum_a[:])
                    psum_2 = p2pool.tile([128, 4 * C_AP], f32, tag="p2")
                    for t4 in range(4):
                        nc.tensor.transpose(out=psum_2[:, C_AP * t4:C_AP * (t4 + 1)],
                                            in_=u[:, 128 * t4:128 * (t4 + 1)],
                                            identity=id_t[:C_AP, :C_AP])
                    nc.vector.tensor_tensor(out=ostage[:, 4 * gg:4 * gg + 4, :],
                                            in0=psum_2[:].rearrange("p (t c) -> p t c", t=4),
                                            in1=bw_t[:].rearrange("p (t c) -> p t c", t=4),
                                            op=mybir.AluOpType.add)
                rows_out = table[sg * SG_ROWS:(sg + 1) * SG_ROWS, :]
                nc.sync.dma_start(
                    out=rows_out.rearrange("(t p) c -> p t c", p=128),
                    in_=ostage[:])
                if table_dbg is not None:
                    dbg_rows = table_dbg[sg * SG_ROWS:(sg + 1) * SG_ROWS, :]
                    nc.sync.dma_start(
                        out=dbg_rows.rearrange("(t p) c -> p t c", p=128),
                        in_=ostage[:])

            # ---------------- phase B: gather the edges ----------------
            # indirect DMA: one descriptor per partition per instruction.
            # pair insts fetch a contiguous run of 2 rows (idx even);
            # single insts fetch 1 row.
            idx_all = gipool.tile([128, K_QUAD + K_PAIR + K_SING], i32)
            nc.sync.dma_start(out=idx_all[:], in_=eidx[:, :])

            def gather_region(k0, n_inst, rows_per, out_off, tag):
                w = rows_per * C_AP
                for b in range(n_inst // GB):
                    g = gopool.tile([128, GB, w], f32, tag=tag)
                    for t in range(GB):
                        k = k0 + b * GB + t
                        nc.gpsimd.indirect_dma_start(
                            out=g[:, t, :],
                            out_offset=None,
                            in_=table[:, :],
                            in_offset=IndirectOffsetOnAxis(
                                ap=idx_all[:, k:k + 1], axis=0),
                        )
                    nc.sync.dma_start(
                        out=out[:, out_off + GB * w * b:out_off + GB * w * (b + 1)],
                        in_=g[:].rearrange("p t c -> p (t c)"),
                    )

            gather_region(0, K_QUAD, 4, 0, "gout4")
            gather_region(K_QUAD, K_PAIR, 2, 4 * K_QUAD * C_AP, "gout2")
            gather_region(K_QUAD + K_PAIR, K_SING, 1,
                          (4 * K_QUAD + 2 * K_PAIR) * C_AP, "gout1")

    nc.compile()
    return nc


def _get_program():
    if "nc" not in _prog_cache:
        _prog_cache["nc"] = _build_program()
    return _prog_cache["nc"]


def kernel(z, ln_gamma, ln_beta, W, flat_atom_res_index, edge_index):
    z = np.asarray(z)
    ln_gamma = np.asarray(ln_gamma, dtype=np.float32)
    ln_beta = np.asarray(ln_beta, dtype=np.float32)
    W = np.asarray(W, dtype=np.float32)
    fari = np.asarray(flat_atom_res_index)
    ei = np.asarray(edge_index)

    n_batch, n_res, _, c_z = z.shape
    assert (n_batch, n_res, c_z) == (1, N_RES, C_Z)
    n_edges = ei.shape[1]

    zf = np.ascontiguousarray(z, dtype=np.float32).reshape(n_batch * n_res * n_res, c_z)

    # ------- host: constants -------
    wg = np.ascontiguousarray((ln_gamma[:, None] * W.T).astype(np.float32))  # [128,16]
    bw = (ln_beta @ W.T).astype(np.float32)                                  # [16]
    bw64 = np.ascontiguousarray(np.tile(bw, (128, 4)).astype(np.float32))    # [128,64]
    ident = np.eye(128, dtype=np.float32)

    # ------- host: bucket edges by r0-block -------
    r0 = fari[ei[0]].astype(np.int64)
    r1 = (fari[ei[1]].astype(np.int64)) % n_res
    core_of = (r0 >> 6).astype(np.int64)          # 64 rows per core
    order = np.argsort(core_of, kind="stable")
    counts = np.bincount(core_of, minlength=N_CORES)
    starts = np.zeros(N_CORES + 1, dtype=np.int64)
    np.cumsum(counts, out=starts[1:])

    r_local = ((r0 & 63) * n_res + r1).astype(np.int32)   # [0, 32768)

    def _take(avail, cap):
        """cap per-block counts so the running total stays <= cap"""
        cs = np.cumsum(avail)
        return np.clip(cap - (cs - avail), 0, avail)

    def _expand(nblk, per_blk):
        tot = int(per_blk.sum())
        K = np.repeat(np.arange(nblk), per_blk)
        st = np.concatenate([[0], np.cumsum(per_blk)[:-1]])
        I = np.arange(tot) - np.repeat(st, per_blk)
        return K, I

    in_maps = []
    quad_ids = []   # per core: list of 4 edge-id arrays
    pair_ids = []   # per core: (pairA, pairB)
    sing_ids = []   # per core: single edge ids
    overflow = []
    for c in range(N_CORES):
        sel = order[starts[c]:starts[c + 1]]
        rows = r_local[sel]
        ordr = np.argsort(rows, kind="stable")
        es = sel[ordr]
        rs = rows[ordr].astype(np.int64)
        cnt = np.bincount(rs, minlength=ROWS_PER_CORE)
        off = np.zeros(ROWS_PER_CORE + 1, dtype=np.int64)
        np.cumsum(cnt, out=off[1:])
        # tier 1: quads over row blocks (4k..4k+3)
        nq = np.minimum.reduce([cnt[0::4], cnt[1::4], cnt[2::4], cnt[3::4]])
        nq = _take(nq, QUAD_CAP)
        tq = int(nq.sum())
        K4, I4 = _expand(ROWS_PER_CORE // 4, nq)
        qE = [es[off[4 * K4 + u] + I4] for u in range(4)]
        quad_ids.append(qE)
        offp = off[:ROWS_PER_CORE] + np.repeat(nq, 4)
        left = cnt - np.repeat(nq, 4)
        # tier 2: pairs over row blocks (2k, 2k+1)
        npk = _take(np.minimum(left[0::2], left[1::2]), PAIR_CAP)
        tp = int(npk.sum())
        K2, I2 = _expand(ROWS_PER_CORE // 2, npk)
        pA = es[offp[2 * K2] + I2]
        pB = es[offp[2 * K2 + 1] + I2]
        pair_ids.append((pA, pB))
        offs = offp + np.repeat(npk, 2)
        lefts = left - np.repeat(npk, 2)
        # tier 3: singles
        ts = int(lefts.sum())
        R, J = _expand(ROWS_PER_CORE, lefts)
        sE = es[offs[R] + J]
        if len(sE) > SING_CAP:
            overflow.append(sE[SING_CAP:])
            sE = sE[:SING_CAP]
        sing_ids.append(sE)
        ibq = np.zeros(QUAD_CAP, dtype=np.int32)
        ibq[:tq] = (4 * K4).astype(np.int32)
        ibp = np.zeros(PAIR_CAP, dtype=np.int32)
        ibp[:tp] = (2 * K2).astype(np.int32)
        ibs = np.zeros(SING_CAP, dtype=np.int32)
        ibs[:len(sE)] = r_local[sE]
        eidx_arr = np.concatenate(
            [ibq.reshape(K_QUAD, 128).T, ibp.reshape(K_PAIR, 128).T,
             ibs.reshape(K_SING, 128).T], axis=1)
        in_maps.append({
            "zs": np.ascontiguousarray(zf[c * ROWS_PER_CORE:(c + 1) * ROWS_PER_CORE]),
            "wg": wg,
            "bw64": bw64,
            "ident": ident,
            "eidx": np.ascontiguousarray(eidx_arr),
        })

    nc = _get_program()
    res = bass_utils.run_bass_kernel_spmd(nc, in_maps, core_ids=list(range(N_CORES)))
    global _LAST_RES
    _LAST_RES = res

    out_full = np.empty((n_edges, C_AP), dtype=np.float32)
    QW = 4 * K_QUAD * C_AP
    PW = 2 * K_PAIR * C_AP
    for c in range(N_CORES):
        dv = res.results[c]["out"]
        Q = dv[:, :QW].reshape(128, K_QUAD, 4, C_AP)
        Q = Q.transpose(1, 0, 2, 3).reshape(QUAD_CAP, 4, C_AP)
        for u in range(4):
            qe = quad_ids[c][u]
            out_full[qe] = Q[:len(qe), u]
        P = dv[:, QW:QW + PW].reshape(128, K_PAIR, 2, C_AP)
        P = P.transpose(1, 0, 2, 3).reshape(PAIR_CAP, 2, C_AP)
        pA, pB = pair_ids[c]
        out_full[pA] = P[:len(pA), 0]
        out_full[pB] = P[:len(pB), 1]
        S = dv[:, QW + PW:].reshape(128, K_SING, C_AP)
        S = S.transpose(1, 0, 2).reshape(SING_CAP, C_AP)
        sE = sing_ids[c]
        out_full[sE] = S[:len(sE)]

    # host fallback for bucket overflow (normally empty)
    for sel in overflow:
        rows = zf[r0[sel] * n_res + r1[sel]].astype(np.float64)
        mu = rows.mean(axis=1, keepdims=True)
        var = rows.var(axis=1)
        xn = (rows - mu) / np.sqrt(var + LN_EPS)[:, None]
        out_full[sel] = (xn @ wg.astype(np.float64) + bw).astype(np.float32)

    return out_full



# revision 9
# speedup vs baseline: 7.3238x; 7.3238x over previous
"""Trainium2 Bass kernel for nn_BroadcastEdgeUpdate.

reference computes:
    res_edge_index = flat_atom_res_index[edge_index]           # [2, E]
    flatish_z      = z.reshape(R, n_res, c_z)                  # R = n_batch*n_res
    update         = einsum('rsc,ac->rsa', LN(flatish_z), W)   # [R, n_res, 16]
    out            = update[res_edge_index[0], res_edge_index[1] % n_res]

Sharding: core i owns table rows r0 in [64*i, 64*i+64) (z first-dim shard);
edges are bucketed by r0-block so each core serves only rows it computes.

Device pipeline per core (8 super-groups of 4096 rows):
  phase A (table build): z slice --DMA--> bn_stats (DVE, 4 groups/inst)
    -> stats combine -> sigma/negmu packed + transposed via PE so that the
    LayerNorm mean/bias terms become rank-1 PSUM-accumulated matmuls:
       P = x @ Wg + (-mu) x g + sigma x bw   (3 matmuls per 128-row chunk,
       out laid [128 rows, 16] so each matmul is only 16 free elems)
       update = P * (1/sigma)                (fused into the PSUM->SBUF copy)
    where Wg = gamma*W^T, bw = beta@W^T, g = colsum(Wg).
  phase B (edge broadcast): the host sorts each core's 32768 table rows by
    per-row edge count (descending) and permutes the z input so the DRAM
    table is written in sorted-rank order.  The j-th copy of each row is
    then a plain DRAM->DRAM prefix copy table[0:N_j] -> out (one slot per
    edge, host un-permutes with a bijective edge -> (pass, rank) map).
    No indirect DMA / Pool engine work at all; prefix copies are issued
    per-super-group as soon as their table range is written, so the copy
    traffic overlaps phase A.
"""

import numpy as np

import concourse.bass as bass
import concourse.bacc as bacc
import concourse.mybir as mybir
import concourse.tile as tile
from concourse import bass_utils

N_CORES = 8
N_RES = 512
C_Z = 128
C_AP = 16
ROWS_PER_CORE = (N_RES // N_CORES) * N_RES  # 32768 table rows
SG_ROWS = 4096
N_SG = ROWS_PER_CORE // SG_ROWS  # 8
LN_EPS = 1e-5
MAX_PASSES = 64  # safety cap; real data needs ~15

_prog_cache = {}


def _build_program(n_pass):
    """n_pass: tuple of prefix lengths (rows) per broadcast pass, shared by
    all cores (max over cores)."""
    f32 = mybir.dt.float32
    nc = bacc.Bacc("TRN2", target_bir_lowering=False, debug=False,
                   num_devices=N_CORES)

    tot_rows = int(sum(n_pass))
    offs = np.concatenate([[0], np.cumsum(n_pass)]).astype(np.int64)

    zs = nc.dram_tensor("zs", [ROWS_PER_CORE, C_Z], f32, kind="ExternalInput").ap()
    wg = nc.dram_tensor("wg", [C_Z, C_AP], f32, kind="ExternalInput").ap()
    # gbig[(u,t'), t, a] = (t'==t) * (g[a] if u==0 else bw[a])
    gbig = nc.dram_tensor("gbig", [64, 32 * C_AP], f32, kind="ExternalInput").ap()
    ident = nc.dram_tensor("ident", [128, 128], f32, kind="ExternalInput").ap()
    out = nc.dram_tensor("out", [tot_rows * C_AP], f32, kind="ExternalOutput").ap()

    with tile.TileContext(nc) as tc:
        with (
            tc.tile_pool(name="const", bufs=1) as cpool,
            tc.tile_pool(name="xin", bufs=3) as xpool,
            tc.tile_pool(name="stat", bufs=2) as spool,
            tc.tile_pool(name="xt", bufs=3) as tpool,
            tc.tile_pool(name="ostage", bufs=2) as opool,
            tc.tile_pool(name="psumT", bufs=3, space="PSUM") as ptpool,
            tc.tile_pool(name="psumU", bufs=2, space="PSUM") as pupool,
            tc.tile_pool(name="psumM", bufs=1, space="PSUM") as pmpool,
            tc.tile_pool(name="tbl", bufs=1, space="DRAM") as dpool,
        ):
            wg_t = cpool.tile([C_Z, C_AP], f32)
            nc.sync.dma_start(out=wg_t[:], in_=wg[:, :])
            gbig_t = cpool.tile([64, 32 * C_AP], f32)
            nc.sync.dma_start(out=gbig_t[:], in_=gbig[:, :])
            id_t = cpool.tile([128, 128], f32)
            nc.sync.dma_start(out=id_t[:], in_=ident[:, :])

            table = dpool.tile([ROWS_PER_CORE, C_AP], f32)

            for sg in range(N_SG):
                lo, hi = sg * SG_ROWS, (sg + 1) * SG_ROWS
                # x[p, t, c] = z row (storage index lo + t*128 + p), which the
                # host arranged to be the row of sorted rank lo + p*32 + t.
                x = xpool.tile([128, 32, C_Z], f32, tag="x")
                nc.sync.dma_start(
                    out=x[:], in_=zs[lo:hi, :].rearrange("(t p) c -> p t c", p=128))

                # --- LayerNorm stats: bn_stats over c, 4 row-groups/inst ---
                stats = spool.tile([128, 32, 6], f32, tag="stats")
                for t in range(32):
                    nc.vector.bn_stats(out=stats[:, t, :], in_=x[:, t, :])
                # combine even/odd half-stats: n=128, halves of 64
                # var = (M2e + M2o + 32*(me-mo)^2)/128 ; mean = (me+mo)/2
                t1 = spool.tile([128, 32, 1], f32, tag="t1")
                t2 = spool.tile([128, 32, 1], f32, tag="t2")
                t3 = spool.tile([128, 32, 1], f32, tag="t3")
                tm = spool.tile([128, 32, 1], f32, tag="tm")
                # musig packs [negmu | sigma] per row -> cols t, 32+t
                musig = spool.tile([128, 64], f32, tag="musig")
                rr = spool.tile([128, 32, 1], f32, tag="rr")
                sig_v = musig[:, 32:64].rearrange("p (t u) -> p t u", u=1)
                neg_v = musig[:, 0:32].rearrange("p (t u) -> p t u", u=1)
                me, mo = stats[:, :, 1:2], stats[:, :, 4:5]
                m2e, m2o = stats[:, :, 2:3], stats[:, :, 5:6]
                nc.vector.tensor_tensor(out=t1[:], in0=me, in1=mo,
                                        op=mybir.AluOpType.subtract)
                nc.vector.tensor_tensor(out=t2[:], in0=m2e, in1=m2o,
                                        op=mybir.AluOpType.add)
                nc.vector.tensor_tensor(out=t3[:], in0=t1[:], in1=t1[:],
                                        op=mybir.AluOpType.mult)
                nc.vector.tensor_scalar(out=t3[:], in0=t3[:], scalar1=32.0,
                                        scalar2=float(C_Z * LN_EPS),
                                        op0=mybir.AluOpType.mult,
                                        op1=mybir.AluOpType.add)
                nc.vector.tensor_tensor(out=t3[:], in0=t3[:], in1=t2[:],
                                        op=mybir.AluOpType.add)
                # sigma = sqrt(var + eps)
                nc.scalar.activation(out=sig_v, in_=t3[:],
                                     func=mybir.ActivationFunctionType.Sqrt,
                                     bias=0.0, scale=1.0 / C_Z)
                nc.vector.reciprocal(out=rr[:], in_=sig_v)
                nc.vector.tensor_tensor(out=tm[:], in0=me, in1=mo,
                                        op=mybir.AluOpType.add)
                nc.vector.tensor_scalar(out=neg_v, in0=tm[:],
                                        scalar1=-0.5, scalar2=None,
                                        op0=mybir.AluOpType.mult)
                # transpose musig -> musigT[{t: negmu, 32+t: sigma}, p]
                pm = pmpool.tile([64, 128], f32, tag="pm")
                nc.tensor.transpose(out=pm[:], in_=musig[:], identity=id_t[:])
                musigT = spool.tile([64, 128], f32, tag="musigT")
                nc.scalar.copy(out=musigT[:], in_=pm[:])

                # --- main path: transpose x, matmul, scale, stage out ---
                ostage = opool.tile([128, 32, C_AP], f32, tag="ostage")
                psum_u = pupool.tile([128, 32, C_AP], f32, tag="pu")
                for gg in range(8):
                    psum_t = ptpool.tile([128, 512], f32, tag="pt")
                    for t4 in range(4):
                        nc.tensor.transpose(
                            out=psum_t[:, 128 * t4:128 * (t4 + 1)],
                            in_=x[:, 4 * gg + t4, :], identity=id_t[:])
                    xt = tpool.tile([128, 512], f32, tag="xt")
                    nc.scalar.copy(out=xt[:], in_=psum_t[:])
                    for j in range(4):
                        t = 4 * gg + j
                        nc.tensor.matmul(
                            out=psum_u[:, t, :],
                            lhsT=xt[:, 128 * j:128 * (j + 1)], rhs=wg_t[:],
                            start=True, stop=False)
                        # rank-1 corrections close the group:
                        # psum_u[m,t,a] += negmu[m,t]*g[a] + sigma[m,t]*bw[a]
                        nc.tensor.matmul(
                            out=psum_u[:, t, :],
                            lhsT=musigT[:], rhs=gbig_t[:, 16 * t:16 * t + 16],
                            start=False, stop=True)
                nc.vector.tensor_tensor(
                    out=ostage[:], in0=psum_u[:],
                    in1=rr[:].broadcast_to((128, 32, C_AP)),
                    op=mybir.AluOpType.mult)

                # table rank r = lo + p*32 + t  (2KB contiguous per partition)
                nc.sync.dma_start(
                    out=table[lo:hi, :].rearrange("(p t) c -> p t c", p=128),
                    in_=ostage[:])

                # --- phase B: broadcast-pass prefix pieces for this sg ---
                for k, nk in enumerate(n_pass):
                    nk = int(nk)
                    if nk <= lo:
                        continue
                    end = min(nk, hi)
                    src = table[lo:end, :].rearrange("r c -> (r c)")
                    dst = out[(int(offs[k]) + lo) * C_AP:(int(offs[k]) + end) * C_AP]
                    nc.sync.dma_start(out=dst, in_=src)

    nc.compile()
    return nc


def _get_program(n_pass=None):
    if n_pass is None:
        if "last" in _prog_cache:
            return _prog_cache["last"]
        n_pass = (ROWS_PER_CORE,)  # degenerate default (unused in practice)
    key = tuple(int(v) for v in n_pass)
    if key not in _prog_cache:
        _prog_cache[key] = _build_program(key)
    _prog_cache["last"] = _prog_cache[key]
    return _prog_cache[key]


def kernel(z, ln_gamma, ln_beta, W, flat_atom_res_index, edge_index):
    z = np.asarray(z)
    ln_gamma = np.asarray(ln_gamma, dtype=np.float32)
    ln_beta = np.asarray(ln_beta, dtype=np.float32)
    W = np.asarray(W, dtype=np.float32)
    fari = np.asarray(flat_atom_res_index)
    ei = np.asarray(edge_index)

    n_batch, n_res, _, c_z = z.shape
    assert (n_batch, n_res, c_z) == (1, N_RES, C_Z)
    n_edges = ei.shape[1]

    zf = np.ascontiguousarray(z, dtype=np.float32).reshape(
        n_batch * n_res * n_res, c_z)

    # ------- host: constants -------
    wg = np.ascontiguousarray((ln_gamma[:, None] * W.T).astype(np.float32))  # [128,16]
    bw = (ln_beta @ W.T).astype(np.float32)                                  # [16]
    gcol = wg.sum(axis=0).astype(np.float32)                                 # [16]
    gbig = np.zeros((64, 32, C_AP), dtype=np.float32)
    for t in range(32):
        gbig[t, t, :] = gcol
        gbig[32 + t, t, :] = bw
    gbig = np.ascontiguousarray(gbig.reshape(64, 32 * C_AP))
    ident = np.eye(128, dtype=np.float32)

    # ------- host: bucket edges by r0-block -------
    r0 = fari[ei[0]].astype(np.int64)
    r1 = (fari[ei[1]].astype(np.int64)) % n_res
    core_of = (r0 >> 6).astype(np.int64)          # 64 rows of r0 per core
    order_e = np.argsort(core_of, kind="stable")
    counts_e = np.bincount(core_of, minlength=N_CORES)
    starts_e = np.zeros(N_CORES + 1, dtype=np.int64)
    np.cumsum(counts_e, out=starts_e[1:])
    r_local = ((r0 & 63) * n_res + r1).astype(np.int64)   # [0, 32768)

    # device processing position: storage i = sg*4096 + t*128 + p holds the
    # row of sorted rank r = sg*4096 + p*32 + t
    rv = np.arange(ROWS_PER_CORE)
    sgv, rem = rv >> 12, rv & 4095
    i_of_rank = (sgv << 12) + ((rem & 31) << 7) + (rem >> 5)

    per_core = []
    n_pass_per_core = []
    for c in range(N_CORES):
        sel = order_e[starts_e[c]:starts_e[c + 1]]
        rows = r_local[sel]
        cnt = np.bincount(rows, minlength=ROWS_PER_CORE)
        rank_to_lr = np.argsort(-cnt, kind="stable")
        rank = np.empty(ROWS_PER_CORE, dtype=np.int64)
        rank[rank_to_lr] = rv
        kmax = int(cnt.max(initial=0))
        nk = np.array([(cnt >= k).sum() for k in range(1, min(kmax, MAX_PASSES) + 1)],
                      dtype=np.int64)
        n_pass_per_core.append(nk)
        per_core.append((sel, rows, cnt, rank_to_lr, rank))

    kmax_all = max((len(nk) for nk in n_pass_per_core), default=1)
    n_pass = tuple(
        int(max((nk[k] if k < len(nk) else 0) for nk in n_pass_per_core))
        for k in range(kmax_all))
    offs = np.concatenate([[0], np.cumsum(n_pass)]).astype(np.int64)
    tot_rows = int(offs[-1])

    in_maps = []
    for c in range(N_CORES):
        sel, rows, cnt, rank_to_lr, rank = per_core[c]
        zslice = zf[c * ROWS_PER_CORE:(c + 1) * ROWS_PER_CORE]
        zr = zslice[rank_to_lr]            # rank order
        zs_host = np.empty_like(zr)
        zs_host[i_of_rank] = zr            # storage order
        in_maps.append({
            "zs": np.ascontiguousarray(zs_host),
            "wg": wg,
            "gbig": gbig,
            "ident": ident,
        })

    nc = _get_program(n_pass)
    res = bass_utils.run_bass_kernel_spmd(nc, in_maps, core_ids=list(range(N_CORES)))
    global _LAST_RES
    _LAST_RES = res

    out_full = np.empty((n_edges, C_AP), dtype=np.float32)
    overflow = []
    for c in range(N_CORES):
        sel, rows, cnt, rank_to_lr, rank = per_core[c]
        dev = np.asarray(res.results[c]["out"]).reshape(tot_rows, C_AP)
        ordr = np.argsort(rows, kind="stable")
        es = sel[ordr]
        rs = rows[ordr]
        n = len(rs)
        if n == 0:
            continue
        first = np.ones(n, dtype=bool)
        first[1:] = rs[1:] != rs[:-1]
        gstart = np.flatnonzero(first)
        gid = np.cumsum(first) - 1
        j = np.arange(n) - gstart[gid]
        ok = j < len(n_pass)
        slots = offs[j[ok]] + rank[rs[ok]]
        out_full[es[ok]] = dev[slots]
        if not ok.all():
            overflow.append((c, es[~ok], rs[~ok]))

    # host fallback for rows hit more than MAX_PASSES times (never on the
    # reference data) -- exact recompute of those few rows
    for c, es_of, rs_of in overflow:
        g0 = (rs_of >> 9) + 64 * c
        g1 = rs_of & (n_res - 1)
        rowsz = zf[g0 * n_res + g1].astype(np.float64)
        mu = rowsz.mean(axis=1, keepdims=True)
        var = rowsz.var(axis=1)
        xn = (rowsz - mu) / np.sqrt(var + LN_EPS)[:, None]
        out_full[es_of] = (xn @ wg.astype(np.float64) + bw).astype(np.float32)

    return out_full
